# revision 12
# baseline (speedup 1.0000x reference)
"""Attention-pooling kernel for 8 Trainium2 NeuronCores.

Reference computation (per batch b):
    h      = tanh(emb @ W + bias)          # [T, 512]
    s      = tanh(h @ c)                   # [T]
    a      = softmax(s)                    # scores in [-1, 1] -> no max-sub needed
    pooled = sum_t a_t * emb[t]            # [512]
    out    = relu(pooled @ w1 + b1) @ w2 + b2

Strategy: pure data-parallel over batch (4 per core).  Host pre-casts the
embedding to bf16 and pre-transposes to [d, t] layout (the matmul
contraction needs d on partitions).  The softmax needs no running max since
scores are tanh-bounded, so a single streaming pass accumulates
exp-weighted sums; the pooled sums stay unnormalized on device and the
host divides by the softmax denominator.

Schedule notes (all from trace analysis):
  - first/last batches use two 512-wide edge tiles so the pipeline primes
    faster and the end-of-kernel dependency chain is shorter
  - W ships as [MO, P, KD, 128] so the mo=0 slice needed by the very first
    matmul is one small contiguous DMA, ahead of the bulk
  - pooling uses the fused DVE tensor_tensor_reduce (mul+reduce in one
    pass), halving DVE busy time and shortening the per-tile chain
  - score tanh is one [1, FT] op per tile (not per 512-half)
  - per-batch pooled reduction keeps the finalize phase off the tail
"""

import os

import numpy as np
import ml_dtypes

B, T, D = 32, 4096, 512
H = 1024
DOUT = 8
NCORES = 8
BL = B // NCORES   # batches per core
P = 128
KD = D // P        # 4 contraction chunks for d=512
MO = D // P        # 4 output chunks for d_out=512

# per-batch compute-tile widths (free/t dim); edge batches use narrow tiles
# at the boundary to shorten pipeline fill and drain
_TILES_FIRST = (512, 512, 1024, 1024, 1024)
_TILES_MID = (1024, 1024, 1024, 1024)
_TILES_LAST = (1024, 1024, 1024, 512, 512)


def _tiles_for_batch(b):
    if b == 0:
        w = _TILES_FIRST
    elif b == BL - 1:
        w = _TILES_LAST
    else:
        w = _TILES_MID
    out = []
    off = 0
    for ft in w:
        out.append((off, ft))
        off += ft
    assert off == T
    return out


NTILES = sum(len(_tiles_for_batch(b)) for b in range(BL))

_last_results = None  # stashed BassKernelResults for test.py profiling


def _build_graph():
    from contextlib import ExitStack

    import concourse.bass as bass
    import concourse.mybir as mybir
    import concourse.tile as tile
    from concourse.bacc import Bacc

    bf16 = mybir.dt.bfloat16
    f32 = mybir.dt.float32
    AF = mybir.ActivationFunctionType
    ALU = mybir.AluOpType

    nc = Bacc(None, target_bir_lowering=False)

    # embedding arrives pre-transposed from the host: emb[b, j, p, t] =
    # embedding[b, t, j*128+p] — so loads are plain contiguous DMAs
    emb = nc.declare_dram_parameter("emb", [BL, KD, P, T], bf16, isOutput=False)
    w_d = nc.declare_dram_parameter("w_sb", [MO, P, KD, P], bf16, isOutput=False)
    c_d = nc.declare_dram_parameter("c_sb", [P, KD], bf16, isOutput=False)
    b_d = nc.declare_dram_parameter("b_sb", [P, MO], f32, isOutput=False)
    w1_d = nc.declare_dram_parameter("w1_sb", [P, KD, H], bf16, isOutput=False)
    w2_d = nc.declare_dram_parameter("w2_sb", [P, H // P, DOUT], bf16, isOutput=False)
    b1r_d = nc.declare_dram_parameter("b1r_sb", [1, H], bf16, isOutput=False)
    out_d = nc.declare_dram_parameter("out", [DOUT, BL], f32, isOutput=True)
    den_d = nc.declare_dram_parameter("den", [1, BL], f32, isOutput=True)

    with tile.TileContext(nc) as tc, ExitStack() as ctx:
        const = ctx.enter_context(tc.tile_pool(name="const", bufs=1))
        embp = ctx.enter_context(tc.tile_pool(name="embp", bufs=2))
        hp = ctx.enter_context(tc.tile_pool(name="hp", bufs=3))
        wp = ctx.enter_context(tc.tile_pool(name="wp", bufs=4))
        scr = ctx.enter_context(tc.tile_pool(name="scr", bufs=4))
        clsp = ctx.enter_context(tc.tile_pool(name="clsp", bufs=1))
        zpsum = ctx.enter_context(tc.tile_pool(name="zpsum", bufs=2, space="PSUM"))
        spsum = ctx.enter_context(tc.tile_pool(name="spsum", bufs=2, space="PSUM"))

        # --- params into SBUF ---
        w_t = const.tile([P, KD, MO, P], bf16)   # [p, ki, mo, col]
        c_t = const.tile([P, KD], bf16)
        b_t = const.tile([P, MO], f32)
        w1_t = const.tile([P, KD, H], bf16)
        b1r_t = const.tile([1, H], bf16)
        w2_t = const.tile([P, H // P, DOUT], bf16)
        # tiny params + the mo=0 weight slice first: the very first matmul
        # (and the first tanh) only need these, so the bulk loads can't
        # delay pipeline start
        nc.scalar.dma_start(out=b_t[:], in_=b_d[:])
        nc.scalar.dma_start(out=c_t[:], in_=c_d[:])
        nc.scalar.dma_start(out=b1r_t[:], in_=b1r_d[:])
        nc.scalar.dma_start(out=w_t[:, :, 0, :], in_=w_d[0])

        # accumulators (written as [P,1] slices, reduced per batch)
        pool_parts = const.tile([P, KD, NTILES], f32)
        denoms = const.tile([1, NTILES], f32)
        pooledn = clsp.tile([P, KD, BL], f32)
        pooled_bf = clsp.tile([P, KD, BL], bf16)

        idx_base = 0
        for b in range(BL):
            tiles = _tiles_for_batch(b)
            embT = embp.tile([P, KD, T], bf16)  # embT[p, j, t] = emb[b, t, j*128+p]
            for (toff, ft) in tiles:
                tsl = slice(toff, toff + ft)
                for j in range(KD):
                    nc.sync.dma_start(
                        out=embT[:, j, tsl],
                        in_=emb[b, j, :, tsl],
                    )
                if b == 0 and toff == 0:
                    # rest of W right behind the first compute tile's data
                    for mo in range(1, MO):
                        nc.scalar.dma_start(out=w_t[:, :, mo, :], in_=w_d[mo])
            if b == 0:
                # classifier weights: needed only at the very end — load them
                # after batch 0's embeddings so they don't steal startup BW
                nc.scalar.dma_start(out=w1_t[:], in_=w1_d[:])
                nc.scalar.dma_start(out=w2_t[:], in_=w2_d[:])
            for ti, (toff, ft) in enumerate(tiles):
                ts = slice(toff, toff + ft)
                nh = ft // 512
                hT = hp.tile([P, MO, ft], bf16, tag="hT")
                for mo in range(MO):
                    zps = zpsum.tile([P, ft], f32, tag="zps")
                    # ki outer / n inner: each W chunk stays stationary for
                    # all 512-col slices (halves LDWEIGHTS traffic)
                    for ki in range(KD):
                        for n in range(nh):
                            ns = slice(n * 512, (n + 1) * 512)
                            tsn = slice(toff + n * 512, toff + (n + 1) * 512)
                            nc.tensor.matmul(
                                zps[:, ns],
                                w_t[:, ki, mo],
                                embT[:, ki, tsn],
                                start=(ki == 0),
                                stop=(ki == KD - 1),
                            )
                    nc.scalar.activation(
                        hT[:, mo], zps[:], AF.Tanh, bias=b_t[:, mo:mo + 1]
                    )
                # scores row: s[0, t] = sum_d c_d * hT[d, t]; the [1, ft]
                # psum spans nh accumulation groups (one per 512-col bank)
                sps = spsum.tile([1, ft], f32, tag="sps")
                for mo in range(MO):
                    for n in range(nh):
                        ns = slice(n * 512, (n + 1) * 512)
                        nc.tensor.matmul(
                            sps[:, ns],
                            c_t[:, mo:mo + 1],
                            hT[:, mo, ns],
                            start=(mo == 0),
                            stop=(mo == MO - 1),
                        )
                st = wp.tile([1, ft], f32, tag="st")
                nc.scalar.activation(st[:], sps[:], AF.Tanh)
                wrow = wp.tile([1, ft], bf16, tag="wrow")
                idx = idx_base + ti
                nc.scalar.activation(
                    wrow[:], st[:], AF.Exp,
                    accum_out=denoms[:1, idx:idx + 1],
                )
                wrep = wp.tile([P, ft], bf16, tag="wrep")
                nc.gpsimd.partition_broadcast(wrep[:], wrow[:])
                for j in range(KD):
                    y = scr.tile([P, ft], bf16, tag="y")
                    nc.vector.tensor_tensor_reduce(
                        out=y[:],
                        in0=embT[:, j, ts],
                        in1=wrep[:],
                        scale=1.0,
                        scalar=0.0,
                        op0=ALU.mult,
                        op1=ALU.add,
                        accum_out=pool_parts[:, j, idx:idx + 1],
                    )
            # fold this batch's tile partials while the next batch computes
            nt = len(tiles)
            nc.vector.tensor_reduce(
                pooledn[:, :, b:b + 1],
                pool_parts[:, :, idx_base:idx_base + nt],
                axis=mybir.AxisListType.X, op=ALU.add,
            )
            nc.vector.tensor_copy(pooled_bf[:, :, b:b + 1], pooledn[:, :, b:b + 1])
            idx_base += nt

        # --- finalize: denominators ---
        dsum = clsp.tile([1, BL], f32)
        off = 0
        for b in range(BL):
            nt = len(_tiles_for_batch(b))
            nc.vector.tensor_reduce(
                dsum[:1, b:b + 1], denoms[:1, off:off + nt],
                axis=mybir.AxisListType.X, op=ALU.add,
            )
            off += nt
        dsum_bf = clsp.tile([1, BL], bf16)
        nc.vector.tensor_copy(dsum_bf[:1], dsum[:1])
        # den leaves now, overlapped with the classifier
        nc.sync.dma_start(out=den_d[:], in_=dsum[:1])

        # --- classifier on UNNORMALIZED pooled sums (host divides by den):
        # relu(num@w1/den + b1) == relu(num@w1 + den*b1)/den, so a K=1 matmul
        # adds den*b1 into the accumulation group ---
        r1 = clsp.tile([P, H // P, BL], bf16)
        ops = zpsum.tile([DOUT, BL], f32, tag="zps")
        for mo in range(H // P):
            c1ps = spsum.tile([P, BL], f32, tag="sps")
            for ki in range(KD):
                nc.tensor.matmul(
                    c1ps[:],
                    w1_t[:, ki, mo * P:(mo + 1) * P],
                    pooled_bf[:, ki, :],
                    start=(ki == 0),
                    stop=False,
                )
            nc.tensor.matmul(
                c1ps[:],
                b1r_t[:1, mo * P:(mo + 1) * P],
                dsum_bf[:1],
                start=False,
                stop=True,
            )
            nc.scalar.activation(r1[:, mo], c1ps[:], AF.Relu)
            # feed this r1 chunk to the output matmul right away so the
            # final accumulation finishes with the relu chain, not after it
            nc.tensor.matmul(
                ops[:],
                w2_t[:, mo, :],
                r1[:, mo, :],
                start=(mo == 0),
                stop=(mo == H // P - 1),
            )
        outsb = clsp.tile([DOUT, BL], f32)
        nc.vector.tensor_copy(outsb[:], ops[:])
        nc.sync.dma_start(out=out_d[:], in_=outsb[:])

    return nc


def kernel(**inputs) -> np.ndarray:
    global _last_results
    from concourse.bass_utils import run_bass_kernel_spmd

    emb = np.asarray(inputs["embedding"], dtype=np.float32)
    W = np.asarray(inputs["weight"], dtype=np.float32)
    bias = np.asarray(inputs["bias"], dtype=np.float32)
    c = np.asarray(inputs["context_weight"], dtype=np.float32)
    w1 = np.asarray(inputs["w1"], dtype=np.float32)
    b1 = np.asarray(inputs["b1"], dtype=np.float32)
    w2 = np.asarray(inputs["w2"], dtype=np.float32)
    b2 = np.asarray(inputs["b2"], dtype=np.float32)

    bf = ml_dtypes.bfloat16
    # pre-transpose on host: [B, T, D] -> [B, KD, P, T] so the device reads
    # contiguous partition rows instead of paying the DMA-xbar transpose
    emb_bf = np.ascontiguousarray(
        emb.astype(bf).reshape(B, T, KD, P).transpose(0, 2, 3, 1))
    # W as [mo, p, ki, col]: the mo=0 slice is one small contiguous DMA
    w_sb = np.ascontiguousarray(
        W.reshape(KD, P, MO, P).transpose(2, 1, 0, 3)).astype(bf)
    c_sb = np.ascontiguousarray(c.reshape(KD, P).T).astype(bf)
    b_sb = np.ascontiguousarray(bias.reshape(MO, P).T).astype(np.float32)
    w1_sb = np.ascontiguousarray(
        w1.reshape(KD, P, H).transpose(1, 0, 2)).astype(bf)
    w2_sb = np.ascontiguousarray(
        w2.reshape(H // P, P, DOUT).transpose(1, 0, 2)).astype(bf)
    b1r_sb = b1.reshape(1, H).astype(bf)

    nc = _build_graph()
    if not nc.is_finalized():
        nc.finalize()
    in_maps = []
    for i in range(NCORES):
        in_maps.append({
            "emb": np.ascontiguousarray(emb_bf[i * BL:(i + 1) * BL]),
            "w_sb": w_sb, "c_sb": c_sb, "b_sb": b_sb,
            "w1_sb": w1_sb, "b1r_sb": b1r_sb, "w2_sb": w2_sb,
        })
    res = run_bass_kernel_spmd(
        nc, in_maps, core_ids=list(range(NCORES)),
        trace=bool(int(os.environ.get("KERNEL_TRACE", "0"))),
    )
    _last_results = res
    parts = []
    for i in range(NCORES):
        pre = np.asarray(res.results[i]["out"], np.float32).T   # [BL, DOUT]
        den = np.asarray(res.results[i]["den"], np.float32)[0]  # [BL]
        parts.append(pre / den[:, None] + b2[None, :])
    return np.concatenate(parts, axis=0).astype(np.float32)
